# revision 30
# baseline (speedup 1.0000x reference)
"""Contrastive-loss (InfoNCE / softmax-CE) kernel for 8 Trainium2 NeuronCores.

reference semantics:
    scores = feature @ anchor.T          # [B, CLS]
    loss   = mean_b( logsumexp(scores[b]) - scores[b, target[b]] )

Strategy (data-parallel, per sharding hint):
  - shard feature/_target along batch across 8 cores (2048 rows each),
    replicate anchor.
  - host prepares transposed layouts (featT [FEAT, BPC] per core, anchorT
    [FEAT, CLS]) so the contraction dim lands on SBUF partitions — pure
    layout prep in the sharding layer.  Matmul inputs are cast to fp8-e4m3
    on the host (fp32 PSUM accumulation; measured ~7e-4 rel on the loss,
    tolerance is 2e-2) which enables DoubleRow perf mode: 2 fp8 MACs per
    PE cell per cycle, contraction 256 per matmul.
  - device pipeline, per core:
      * anchor/feature DMAs interleaved per contraction chunk (doubling
        sizes) on both HWDGE rings so the first matmul starts ~1us in
        instead of after the full anchor load;
      * PE: kt-outer DoubleRow matmuls in groups of 2 batch tiles (4 PSUM
        banks), double-buffered against the other 4 banks so softmax
        post-processing never stalls the PE; the last two groups are
        single-tile so the final chain overlaps the preceding matmuls;
      * per tile: row max (DVE) -> exp with fused sum (ACT, Exp table stays
        resident) -> target score via iota==target mask (DVE, fused accum);
      * one [128, 16]-column stats DMA per quantity (-max, sum-exp,
        target-score) in SBUF-mirroring DRAM layout.
  - host: nll = ln(sume) - nmx - st, then mean over all 16384 rows (the
    scalar all-reduce).

Matmul dtype knob (CL_MM_DTYPE): fp8 (default, DoubleRow), bf16, f32r, f32.
"""

import os
import sys
from contextlib import ExitStack

import numpy as np

for _p in ("/opt/trn_rl_repo",):
    if os.path.isdir(_p) and _p not in sys.path:
        sys.path.insert(0, _p)

import concourse.bass as bass
import concourse.bacc as bacc
import concourse.mybir as mybir
import concourse.tile as tile

B, CLS, FEAT = 16384, 1000, 2048
NCORES = 8
BPC = B // NCORES          # 2048 batch rows per core
P = 128                    # partitions
KT = FEAT // P             # 16 contraction tiles
MT = BPC // P              # 16 batch tiles per core
GRP = 2                    # m-tiles per PSUM group (4 banks -> double-buffer)
NGRP = MT // GRP           # 8 groups
SLAB = 4                   # m-tiles per feature slab (DMA batching)
NSLAB = MT // SLAB         # 4 slabs
N0 = 512                   # first class tile (one PSUM bank of fp32)

MM_DTYPE = os.environ.get("CL_MM_DTYPE", "fp8")


def _mm_dt(mm_dtype: str):
    return {
        "f32": mybir.dt.float32,
        "f32r": mybir.dt.float32r,
        "bf16": mybir.dt.bfloat16,
        "fp8": mybir.dt.float8e4,
    }[mm_dtype]


def _clsp(mm_dtype: str) -> int:
    # fp8 DoubleRow needs the weights/moving Ko-step to be a multiple of 16
    # bytes, so the class dim is padded 1000 -> 1008 with zero anchor columns
    # (score 0 << row max ~140, and targets never hit the pad classes)
    return 1008 if mm_dtype == "fp8" else CLS


def build_program(mm_dtype: str = MM_DTYPE, reps: int = 1,
                  loop_iters: int = 1) -> bass.Bass:
    """Build the per-core Bass/Tile program (SPMD: same program on all cores).

    reps > 1 repeats the full body (including all DMAs) for differential
    device-time measurement; loop_iters > 1 does the same with a hardware
    For_i loop (compact code, ~2us back-edge per iteration)."""
    f32 = mybir.dt.float32
    mdt = _mm_dt(mm_dtype)

    clsp = _clsp(mm_dtype)
    nc = bacc.Bacc(None, target_bir_lowering=False, debug=False)
    featT = nc.dram_tensor("featT", [FEAT, BPC], mdt, kind="ExternalInput")
    anchorT = nc.dram_tensor("anchorT", [FEAT, clsp], mdt, kind="ExternalInput")
    tgt = nc.dram_tensor("tgt", [BPC], f32, kind="ExternalInput")
    # per-m-tile stats, SBUF-mirroring layout [P, 3*MT]: row p, col s*MT+m
    # holds stat s of batch row m*128+p; the host finishes
    # nll = ln(sume) - nmx - st and takes the mean
    nll = nc.dram_tensor("nll", [P, 3 * MT], f32, kind="ExternalOutput")

    # [p, kt, m] / [p, kt, c] views with the contraction dim on partitions
    fview = featT.ap().rearrange("(kt p) m -> p kt m", p=P)   # [128, 16, 2048]
    aview = anchorT.ap().rearrange("(kt p) c -> p kt c", p=P)  # [128, 16, 1000]

    slab_bufs = 3 if mm_dtype == "bf16" else 2

    with tile.TileContext(nc) as tc, ExitStack() as ctx:
        singles = ctx.enter_context(tc.tile_pool(name="singles", bufs=1))
        feats = ctx.enter_context(tc.tile_pool(name="feats", bufs=slab_bufs))
        psum = ctx.enter_context(tc.tile_pool(name="psum", bufs=4, space="PSUM"))
        scratch = ctx.enter_context(tc.tile_pool(name="scratch", bufs=4))

        # iota row 0..clsp-1 (exact in f32), replicated on every partition
        iota_i = singles.tile([P, clsp], mybir.dt.int32)
        nc.gpsimd.iota(iota_i, pattern=[[1, clsp]], base=0, channel_multiplier=0)
        iota_f = singles.tile([P, clsp], f32)
        nc.vector.tensor_copy(out=iota_f, in_=iota_i)

        if loop_iters > 1:
            assert reps == 1
            with tc.For_i(0, loop_iters, 1):
                _loss_body(nc, tc, mm_dtype, fview, aview, tgt, nll,
                           iota_f, singles, feats, psum, scratch)
        else:
            for _rep in range(reps):
                _loss_body(nc, tc, mm_dtype, fview, aview, tgt, nll,
                           iota_f, singles, feats, psum, scratch)

    return nc


def _chunk_splits(mm_dtype):
    """Doubling kt splits for the head DMAs: fine-grained first chunks so the
    kt-outer matmul sweep of group 0 starts ~1us in, coarse tail chunks to
    keep the HWDGE descriptor-generation count low.  fp8 DoubleRow consumes
    kt chunks in pairs, so splits stay pair-aligned there."""
    sizes = (2, 2, 4, 8) if mm_dtype == "fp8" else (1, 1, 2, 4, 8)
    out, k = [], 0
    for sz in sizes:
        out.append((k, k + sz))
        k += sz
    assert k == KT
    return out


def _loss_body(nc, tc, mm_dtype, fview, aview, tgt, nll, iota_f,
               singles, feats, psum, scratch):
    f32 = mybir.dt.float32
    mdt = _mm_dt(mm_dtype)
    clsp = _clsp(mm_dtype)
    n1 = clsp - N0
    fp8 = mm_dtype == "fp8"

    anchor_sb = singles.tile([P, KT, clsp], mdt, name="anchor_sb")
    slabs = [None] * NSLAB

    def load_slab(s, splits=((0, KT),), engine=None):
        # prologue slabs ride the scalar-engine HWDGE ring so their
        # descriptor generation runs in parallel with the anchor DMAs on the
        # sync ring; in-loop prefetches stay on sync (the ACT queue is busy
        # with exp ops by then)
        slabs[s] = feats.tile([P, KT, SLAB * P], mdt, name="slab", tag="slab")
        for k0, k1 in splits:
            (engine or nc.sync).dma_start(
                out=slabs[s][:, k0:k1, :],
                in_=fview[:, k0:k1, s * SLAB * P : (s + 1) * SLAB * P],
            )

    # interleaved anchor/slab0 head chunks in consumption order
    slabs[0] = feats.tile([P, KT, SLAB * P], mdt, name="slab", tag="slab")
    tgt_sb = singles.tile([P, MT], f32, name="tgt_sb")
    for i, (k0, k1) in enumerate(_chunk_splits(mm_dtype)):
        nc.sync.dma_start(out=anchor_sb[:, k0:k1, :], in_=aview[:, k0:k1, :])
        nc.scalar.dma_start(
            out=slabs[0][:, k0:k1, :], in_=fview[:, k0:k1, 0 : SLAB * P]
        )
        if i == 0:
            # per-row target index as f32; column m holds rows [m*128, ..)
            nc.sync.dma_start(
                out=tgt_sb, in_=tgt.ap().rearrange("(m p) -> p m", p=P)
            )
    load_slab(1, engine=nc.scalar)

    # PE warm-up during the DMA head: ~3.5us of dummy matmuls on a
    # zero-filled SBUF tile releases the HAM clock-gate (4096-cycle activity
    # window) so the real matmuls start at 2.4 GHz instead of 1.2 GHz.
    # They write into group 0's first PSUM bank; the real kt=0 matmul's
    # start=True clears the bank's has_written bits, so results are exact.
    warm = singles.tile([P, P], mdt, name="warm")
    nc.vector.memset(warm, 0)
    warm_ps = psum.tile([P, 2, N0], f32, name="ps", tag="ps")
    for _ in range(48):
        nc.tensor.matmul(
            warm_ps[:, 0, 0:64], warm, warm[:, 0:64], start=True, stop=True,
            skip_group_check=True,
        )

    # per-m-tile stats, shipped to the host which finishes
    # nll = ln(sume) - nmx - st (and the mean).  Separate tiles so the
    # cross-engine reads/writes of different stats don't false-serialize.
    nmx_all = singles.tile([P, MT], f32, name="nmx_all")    # -max per row
    sume_all = singles.tile([P, MT], f32, name="sume_all")  # sum exp(s - max)
    st_all = singles.tile([P, MT], f32, name="st_all")      # score[target]

    # groups of 2 m-tiles (4 PSUM banks, double-buffered), except the last
    # two m-tiles run as single-tile groups so the final post-processing
    # chain overlaps the preceding matmuls
    groups = [(m, GRP) for m in range(0, MT - 2, GRP)] + [(MT - 2, 1), (MT - 1, 1)]

    for m0, gsz in groups:
        s = m0 // SLAB
        if s + 2 < NSLAB and slabs[s + 2] is None:
            load_slab(s + 2)
        slab = slabs[s]
        off = (m0 % SLAB) * P

        # kt-outer over the group's m-tiles: each arriving anchor/slab
        # chunk unlocks the group's matmuls, so PE saturates while later
        # chunks are still streaming in.
        ps_list = [
            psum.tile([P, 2, N0], f32, name="ps", tag="ps")
            for _ in range(gsz)
        ]
        if fp8:
            # DoubleRow: adjacent kt chunks pair up as the Ko=2 axis of both
            # operands (contraction 256 per matmul, 2 MACs/cell/cycle)
            dr = mybir.MatmulPerfMode.DoubleRow
            for kk in range(KT // 2):
                for mi in range(gsz):
                    msl = slice(off + mi * P, off + (mi + 1) * P)
                    nc.tensor.matmul(
                        ps_list[mi][:, 0, :],
                        slab[:, 2 * kk : 2 * kk + 2, msl],
                        anchor_sb[:, 2 * kk : 2 * kk + 2, 0:N0],
                        start=(kk == 0),
                        stop=(kk == KT // 2 - 1),
                        perf_mode=dr,
                    )
                    nc.tensor.matmul(
                        ps_list[mi][:, 1, 0:n1],
                        slab[:, 2 * kk : 2 * kk + 2, msl],
                        anchor_sb[:, 2 * kk : 2 * kk + 2, N0:clsp],
                        start=(kk == 0),
                        stop=(kk == KT // 2 - 1),
                        perf_mode=dr,
                    )
        else:
            for kt in range(KT):
                for mi in range(gsz):
                    msl = slice(off + mi * P, off + (mi + 1) * P)
                    nc.tensor.matmul(
                        ps_list[mi][:, 0, :],
                        slab[:, kt, msl],
                        anchor_sb[:, kt, 0:N0],
                        start=(kt == 0),
                        stop=(kt == KT - 1),
                    )
                    nc.tensor.matmul(
                        ps_list[mi][:, 1, 0:n1],
                        slab[:, kt, msl],
                        anchor_sb[:, kt, N0:clsp],
                        start=(kt == 0),
                        stop=(kt == KT - 1),
                    )

        for mi in range(gsz):
            m = m0 + mi
            ps = ps_list[mi]
            # flat[:, 0:clsp] covers bank0[0:512] + bank1[0:clsp-512]: exactly
            # the computed score columns, so no tail padding is needed
            flat = ps.rearrange("p a b -> p (a b)")
            valid = flat[:, 0:clsp]

            nc.vector.tensor_reduce(
                out=nmx_all[:, m : m + 1],
                in_=valid,
                axis=mybir.AxisListType.X,
                op=mybir.AluOpType.max,
                negate=True,
            )

            # exp(scores - max) with fused per-row sum on the ACT engine;
            # Exp is the only ACT function used -> its table loads once
            expt = scratch.tile([P, clsp], f32, name="expt")
            nc.scalar.activation(
                out=expt,
                in_=valid,
                func=mybir.ActivationFunctionType.Exp,
                bias=nmx_all[:, m : m + 1],
                scale=1.0,
                accum_out=sume_all[:, m : m + 1],
            )

            # s_target = sum_c scores[c] * (iota[c] == target), one DVE pass
            junk = scratch.tile([P, clsp], f32, name="junk")
            nc.vector.scalar_tensor_tensor(
                out=junk,
                in0=iota_f,
                scalar=tgt_sb[:, m : m + 1],
                in1=valid,
                op0=mybir.AluOpType.is_equal,
                op1=mybir.AluOpType.mult,
                accum_out=st_all[:, m : m + 1],
            )

    out_ap = nll.ap()
    nc.sync.dma_start(out=out_ap[:, 0 * MT : 1 * MT], in_=nmx_all)
    nc.sync.dma_start(out=out_ap[:, 1 * MT : 2 * MT], in_=sume_all)
    nc.sync.dma_start(out=out_ap[:, 2 * MT : 3 * MT], in_=st_all)


def _np_mm(mm_dtype: str):
    if mm_dtype in ("bf16", "fp8"):
        import ml_dtypes

        return np.dtype(
            ml_dtypes.bfloat16 if mm_dtype == "bf16" else ml_dtypes.float8_e4m3
        )
    return np.dtype(np.float32)


def prepare_inputs(feature, anchor, _target, mm_dtype: str = MM_DTYPE):
    """Host-side sharding + layout prep. Returns per-core input maps."""
    npdt = _np_mm(mm_dtype)
    clsp = _clsp(mm_dtype)
    feature = np.asarray(feature, dtype=np.float32)
    anchor = np.asarray(anchor, dtype=np.float32)
    tgt_f = np.asarray(_target).astype(np.float32)

    anchorT = np.zeros((FEAT, clsp), dtype=npdt)
    anchorT[:, :CLS] = anchor.T.astype(npdt)  # pad classes stay all-zero
    in_maps = []
    for c in range(NCORES):
        sl = slice(c * BPC, (c + 1) * BPC)
        featT_c = np.ascontiguousarray(feature[sl].T.astype(npdt))  # [FEAT, BPC]
        in_maps.append(
            {
                "featT": featT_c,
                "anchorT": anchorT,
                "tgt": np.ascontiguousarray(tgt_f[sl]),
            }
        )
    return in_maps


_PROGRAM_CACHE: dict = {}


def _get_program(mm_dtype: str, reps: int = 1, variant: str = "hostT") -> bass.Bass:
    key = (mm_dtype, reps, variant)
    nc = _PROGRAM_CACHE.get(key)
    if nc is None:
        if variant == "hostT":
            nc = build_program(mm_dtype, reps=reps)
        elif variant.startswith("loop"):
            nc = build_program(mm_dtype, loop_iters=int(variant[4:]))
        else:
            raise ValueError(variant)
        nc.compile()  # bacc pass pipeline (reg alloc, wait splitting, ...)
        _PROGRAM_CACHE[key] = nc
    return nc


_RUNNER_CACHE: dict = {}


def make_runner(nc: bass.Bass, in_maps):
    """Compile once; return callable that re-executes with device-resident
    inputs (only the tiny donated output zeros are re-created per call)."""
    import jax
    import jax.core
    from jax.experimental.shard_map import shard_map
    from jax.sharding import Mesh, NamedSharding, PartitionSpec

    from concourse import bass2jax, mybir as mb

    bass2jax.install_neuronx_cc_hook()

    partition_name = (
        nc.partition_id_tensor.name if nc.partition_id_tensor else None
    )
    in_names, out_names, out_avals, zero_shapes = [], [], [], []
    for alloc in nc.m.functions[0].allocations:
        if not isinstance(alloc, mb.MemoryLocationSet):
            continue
        name = alloc.memorylocations[0].name
        if alloc.kind == "ExternalInput":
            if name != partition_name:
                in_names.append(name)
        elif alloc.kind == "ExternalOutput":
            shape = tuple(alloc.tensor_shape)
            dtype = mb.dt.np(alloc.dtype)
            out_names.append(name)
            out_avals.append(jax.core.ShapedArray(shape, dtype))
            zero_shapes.append((shape, dtype))
    n_params = len(in_names)
    n_outs = len(out_names)
    all_in_names = list(in_names) + list(out_names)
    if partition_name is not None:
        all_in_names.append(partition_name)

    donate = tuple(range(n_params, n_params + n_outs))

    def _body(*args):
        operands = list(args)
        if partition_name is not None:
            operands.append(bass2jax.partition_id_tensor())
        outs = bass2jax._bass_exec_p.bind(
            *operands,
            out_avals=tuple(out_avals),
            in_names=tuple(all_in_names),
            out_names=tuple(out_names),
            lowering_input_output_aliases=(),
            sim_require_finite=True,
            sim_require_nnan=True,
            nc=nc,
        )
        return tuple(outs)

    devices = jax.devices()[:NCORES]
    mesh = Mesh(np.asarray(devices), ("core",))
    in_specs = (PartitionSpec("core"),) * (n_params + n_outs)
    out_specs = (PartitionSpec("core"),) * n_outs
    sharded = jax.jit(
        shard_map(
            _body, mesh=mesh, in_specs=in_specs, out_specs=out_specs,
            check_rep=False,
        ),
        donate_argnums=donate,
        keep_unused=True,
    )
    sharding = NamedSharding(mesh, PartitionSpec("core"))
    dev_in = [
        jax.device_put(
            np.concatenate([np.asarray(in_maps[c][nm]) for c in range(NCORES)], axis=0),
            sharding,
        )
        for nm in in_names
    ]
    jax.block_until_ready(dev_in)

    def run():
        zeros = [
            np.zeros((NCORES * s[0], *s[1:]), dt) for (s, dt) in zero_shapes
        ]
        outs = sharded(*dev_in, *zeros)
        jax.block_until_ready(outs)
        return {
            nm: np.asarray(outs[i]).reshape(NCORES, *out_avals[i].shape)
            for i, nm in enumerate(out_names)
        }

    return run


def timed_run(in_maps, mm_dtype: str = MM_DTYPE, reps: int = 1, iters: int = 3,
              variant: str = "hostT"):
    """Compile the reps-times-repeated program, return best wall seconds/call."""
    import time

    key = (mm_dtype, reps, variant, id(in_maps))
    runner = _RUNNER_CACHE.get(key)
    if runner is None:
        nc = _get_program(mm_dtype, reps=reps, variant=variant)
        runner = make_runner(nc, in_maps)
        _RUNNER_CACHE[key] = runner
    runner()  # warmup (compile + first exec)
    best = float("inf")
    for _ in range(iters):
        t0 = time.perf_counter()
        runner()
        best = min(best, time.perf_counter() - t0)
    return best


def run_on_cores(in_maps, mm_dtype: str = MM_DTYPE, trace: bool = False):
    from concourse.bass_utils import run_bass_kernel_spmd

    nc = _get_program(mm_dtype)
    res = run_bass_kernel_spmd(nc, in_maps, list(range(NCORES)), trace=trace)
    return res


def finish_nll(stats: np.ndarray) -> np.ndarray:
    """Per-core [P, 3*MT] device stats -> per-row nll [BPC] (float64)."""
    s = stats.astype(np.float64).reshape(P, 3, MT)
    nmx = s[:, 0, :].T.ravel()   # batch row m*128+p <- [p, m]
    sume = s[:, 1, :].T.ravel()
    st = s[:, 2, :].T.ravel()
    return np.log(sume) - nmx - st


def kernel(feature, anchor, _target) -> np.ndarray:
    mm_dtype = MM_DTYPE
    in_maps = prepare_inputs(feature, anchor, _target, mm_dtype)
    res = run_on_cores(in_maps, mm_dtype, trace=os.environ.get("CL_TRACE", "") == "1")
    nll_all = np.concatenate(
        [finish_nll(res.results[c]["nll"]) for c in range(NCORES)]
    )
    if os.environ.get("CL_TRACE", "") == "1" and res.exec_time_ns is not None:
        print(f"HW exec time: {res.exec_time_ns} ns")
    return np.asarray(np.mean(nll_all, dtype=np.float64), dtype=np.float32)


# revision 43
# speedup vs baseline: 1.0397x; 1.0397x over previous
"""Contrastive-loss (InfoNCE / softmax-CE) kernel for 8 Trainium2 NeuronCores.

reference semantics:
    scores = feature @ anchor.T          # [B, CLS]
    loss   = mean_b( logsumexp(scores[b]) - scores[b, target[b]] )

Strategy (data-parallel, per sharding hint):
  - shard feature/_target along batch across 8 cores (2048 rows each),
    replicate anchor.
  - host prepares transposed layouts (featT [FEAT, BPC] per core, anchorT
    [FEAT, CLS]) so the contraction dim lands on SBUF partitions — pure
    layout prep in the sharding layer.  Matmul inputs are cast to fp8-e4m3
    on the host (fp32 PSUM accumulation; measured ~7e-4 rel on the loss,
    tolerance is 2e-2) which enables DoubleRow perf mode: 2 fp8 MACs per
    PE cell per cycle, contraction 256 per matmul.
  - device pipeline, per core:
      * anchor/feature DMAs interleaved per contraction chunk (doubling
        sizes) on both HWDGE rings so the first matmul starts ~1us in
        instead of after the full anchor load;
      * PE: kt-outer DoubleRow matmuls in groups of 2 batch tiles (4 PSUM
        banks), double-buffered against the other 4 banks so softmax
        post-processing never stalls the PE; the last two groups are
        single-tile so the final chain overlaps the preceding matmuls;
      * per tile: row max (DVE) -> exp with fused sum (ACT, Exp table stays
        resident) -> target score via iota==target mask (DVE, fused accum);
      * one [128, 16]-column stats DMA per quantity (-max, sum-exp,
        target-score) in SBUF-mirroring DRAM layout.
  - host: nll = ln(sume) - nmx - st, then mean over all 16384 rows (the
    scalar all-reduce).

Matmul dtype knob (CL_MM_DTYPE): fp8 (default, DoubleRow), bf16, f32r, f32.
"""

import os
import sys
from contextlib import ExitStack

import numpy as np

for _p in ("/opt/trn_rl_repo",):
    if os.path.isdir(_p) and _p not in sys.path:
        sys.path.insert(0, _p)

import concourse.bass as bass
import concourse.bacc as bacc
import concourse.mybir as mybir
import concourse.tile as tile

B, CLS, FEAT = 16384, 1000, 2048
NCORES = 8
BPC = B // NCORES          # 2048 batch rows per core
P = 128                    # partitions
KT = FEAT // P             # 16 contraction tiles
MT = BPC // P              # 16 batch tiles per core
GRP = 2                    # m-tiles per PSUM group (4 banks -> double-buffer)
NGRP = MT // GRP           # 8 groups
SLAB = 4                   # m-tiles per feature slab (DMA batching)
NSLAB = MT // SLAB         # 4 slabs
N0 = 512                   # first class tile (one PSUM bank of fp32)

MM_DTYPE = os.environ.get("CL_MM_DTYPE", "fp8")


def _mm_dt(mm_dtype: str):
    return {
        "f32": mybir.dt.float32,
        "f32r": mybir.dt.float32r,
        "bf16": mybir.dt.bfloat16,
        "fp8": mybir.dt.float8e4,
    }[mm_dtype]


def _clsp(mm_dtype: str) -> int:
    # fp8 DoubleRow needs the operands' Ko-step (the anchor row pitch) to be
    # a multiple of 16 bytes, so the stored class dim is padded 1000 -> 1008;
    # the pad columns are never loaded into a matmul or the softmax
    return 1008 if mm_dtype == "fp8" else CLS


def build_program(mm_dtype: str = MM_DTYPE, reps: int = 1,
                  loop_iters: int = 1) -> bass.Bass:
    """Build the per-core Bass/Tile program (SPMD: same program on all cores).

    reps > 1 repeats the full body (including all DMAs) for differential
    device-time measurement; loop_iters > 1 does the same with a hardware
    For_i loop (compact code, ~2us back-edge per iteration)."""
    f32 = mybir.dt.float32
    mdt = _mm_dt(mm_dtype)

    clsp = _clsp(mm_dtype)
    nc = bacc.Bacc(None, target_bir_lowering=False, debug=False)
    featT = nc.dram_tensor("featT", [FEAT, BPC], mdt, kind="ExternalInput")
    anchorT = nc.dram_tensor("anchorT", [FEAT, clsp], mdt, kind="ExternalInput")
    tgt = nc.dram_tensor("tgt", [BPC], f32, kind="ExternalInput")
    # per-m-tile stats, SBUF-mirroring layout [P, 3*MT]: row p, col s*MT+m
    # holds stat s of batch row m*128+p; the host finishes
    # nll = ln(sume) - nmx - st and takes the mean
    nll = nc.dram_tensor("nll", [P, 3 * MT], f32, kind="ExternalOutput")

    # [p, kt, m] / [p, kt, c] views with the contraction dim on partitions
    fview = featT.ap().rearrange("(kt p) m -> p kt m", p=P)   # [128, 16, 2048]
    aview = anchorT.ap().rearrange("(kt p) c -> p kt c", p=P)  # [128, 16, 1000]

    # triple-buffer the 16/8KB-per-partition bf16/fp8 slabs (true 2-ahead
    # prefetch); the 32KB f32 slabs only fit double-buffered
    slab_bufs = 2 if mm_dtype in ("f32", "f32r") else 3

    with tile.TileContext(nc) as tc, ExitStack() as ctx:
        singles = ctx.enter_context(tc.tile_pool(name="singles", bufs=1))
        feats = ctx.enter_context(tc.tile_pool(name="feats", bufs=slab_bufs))
        psum = ctx.enter_context(tc.tile_pool(name="psum", bufs=4, space="PSUM"))
        scratch = ctx.enter_context(tc.tile_pool(name="scratch", bufs=4))

        # iota row 0..CLS-1 (exact in f32), replicated on every partition
        iota_i = singles.tile([P, CLS], mybir.dt.int32)
        nc.gpsimd.iota(iota_i, pattern=[[1, CLS]], base=0, channel_multiplier=0)
        iota_f = singles.tile([P, CLS], f32)
        nc.vector.tensor_copy(out=iota_f, in_=iota_i)

        # PE warm-up during the DMA head (once per program, not per loop
        # iteration): ~2.5us of dummy matmuls on a zero-filled SBUF tile
        # releases the HAM clock-gate (4096-cycle activity window) so the
        # real matmuls start at 2.4 GHz instead of 1.2 GHz.  They write into
        # one PSUM bank; every real accumulation's first matmul runs with
        # start=True which clears the bank's has_written bits, so results
        # stay exact.
        warm = singles.tile([P, P], mdt, name="warm")
        nc.vector.memset(warm, 0)
        warm_ps = psum.tile([P, 2, N0], mybir.dt.float32, name="ps", tag="ps")
        for _ in range(48):
            nc.tensor.matmul(
                warm_ps[:, 0, 0:64], warm, warm[:, 0:64], start=True,
                stop=True, skip_group_check=True,
            )

        if loop_iters > 1:
            assert reps == 1
            with tc.For_i(0, loop_iters, 1):
                _loss_body(nc, tc, mm_dtype, fview, aview, tgt, nll,
                           iota_f, singles, feats, psum, scratch)
        else:
            for _rep in range(reps):
                _loss_body(nc, tc, mm_dtype, fview, aview, tgt, nll,
                           iota_f, singles, feats, psum, scratch)

    return nc


def _chunk_splits(mm_dtype):
    """kt splits for the head DMAs: fine-grained chunks so the kt-outer
    matmul sweep of group 0 tracks the arrival stream, slightly coarser at
    the tail to keep the HWDGE descriptor-generation count low.  fp8
    DoubleRow consumes kt chunks in pairs, so splits stay pair-aligned."""
    sizes = (2, 2, 2, 2, 4, 4) if mm_dtype == "fp8" else (1, 1, 2, 4, 8)
    out, k = [], 0
    for sz in sizes:
        out.append((k, k + sz))
        k += sz
    assert k == KT
    return out


def _loss_body(nc, tc, mm_dtype, fview, aview, tgt, nll, iota_f,
               singles, feats, psum, scratch):
    f32 = mybir.dt.float32
    mdt = _mm_dt(mm_dtype)
    clsp = _clsp(mm_dtype)
    # the pad columns exist only for the DRAM/SBUF row pitch (Ko-step rule);
    # the matmuls and softmax cover the 1000 real classes
    n1 = CLS - N0
    fp8 = mm_dtype == "fp8"

    anchor_sb = singles.tile([P, KT, clsp], mdt, name="anchor_sb")
    slabs = [None] * NSLAB

    def load_slab(s, splits=((0, KT),), engine=None):
        # prologue slabs ride the scalar-engine HWDGE ring so their
        # descriptor generation runs in parallel with the anchor DMAs on the
        # sync ring; in-loop prefetches stay on sync (the ACT queue is busy
        # with exp ops by then)
        slabs[s] = feats.tile([P, KT, SLAB * P], mdt, name="slab", tag="slab")
        for k0, k1 in splits:
            (engine or nc.sync).dma_start(
                out=slabs[s][:, k0:k1, :],
                in_=fview[:, k0:k1, s * SLAB * P : (s + 1) * SLAB * P],
            )

    # interleaved anchor/slab0 head chunks in consumption order; slab0 is
    # split by column halves (group 0 reads only cols [0:256], so its half
    # streams first and the first-group working set shrinks to 2.5 MB)
    slabs[0] = feats.tile([P, KT, SLAB * P], mdt, name="slab", tag="slab")
    tgt_sb = singles.tile([P, MT], f32, name="tgt_sb")
    for i, (k0, k1) in enumerate(_chunk_splits(mm_dtype)):
        nc.sync.dma_start(out=anchor_sb[:, k0:k1, :], in_=aview[:, k0:k1, :])
        nc.scalar.dma_start(
            out=slabs[0][:, k0:k1, 0 : GRP * P],
            in_=fview[:, k0:k1, 0 : GRP * P],
        )
        if i == 0:
            # per-row target index as f32; column m holds rows [m*128, ..)
            nc.sync.dma_start(
                out=tgt_sb, in_=tgt.ap().rearrange("(m p) -> p m", p=P)
            )
    nc.scalar.dma_start(
        out=slabs[0][:, :, GRP * P : SLAB * P],
        in_=fview[:, :, GRP * P : SLAB * P],
    )
    load_slab(1, engine=nc.scalar)

    # per-m-tile stats, shipped to the host which finishes
    # nll = ln(sume) - nmx - st (and the mean).  Separate tiles so the
    # cross-engine reads/writes of different stats don't false-serialize.
    nmx_all = singles.tile([P, MT], f32, name="nmx_all")    # -max per row
    sume_all = singles.tile([P, MT], f32, name="sume_all")  # sum exp(s - max)
    st_all = singles.tile([P, MT], f32, name="st_all")      # score[target]

    # groups of 2 m-tiles (4 PSUM banks, double-buffered), except the last
    # two m-tiles run as single-tile groups so the final post-processing
    # chain overlaps the preceding matmuls
    groups = [(m, GRP) for m in range(0, MT - 2, GRP)] + [(MT - 2, 1), (MT - 1, 1)]

    for m0, gsz in groups:
        s = m0 // SLAB
        if s + 2 < NSLAB and slabs[s + 2] is None:
            load_slab(s + 2)
        slab = slabs[s]
        off = (m0 % SLAB) * P

        # kt-outer over the group's m-tiles: each arriving anchor/slab
        # chunk unlocks the group's matmuls, so PE saturates while later
        # chunks are still streaming in.
        ps_list = [
            psum.tile([P, 2, N0], f32, name="ps", tag="ps")
            for _ in range(gsz)
        ]
        if fp8:
            # DoubleRow: adjacent kt chunks pair up as the Ko=2 axis of both
            # operands (contraction 256 per matmul, 2 MACs/cell/cycle)
            dr = mybir.MatmulPerfMode.DoubleRow
            for kk in range(KT // 2):
                for mi in range(gsz):
                    msl = slice(off + mi * P, off + (mi + 1) * P)
                    nc.tensor.matmul(
                        ps_list[mi][:, 0, :],
                        slab[:, 2 * kk : 2 * kk + 2, msl],
                        anchor_sb[:, 2 * kk : 2 * kk + 2, 0:N0],
                        start=(kk == 0),
                        stop=(kk == KT // 2 - 1),
                        perf_mode=dr,
                    )
                    nc.tensor.matmul(
                        ps_list[mi][:, 1, 0:n1],
                        slab[:, 2 * kk : 2 * kk + 2, msl],
                        anchor_sb[:, 2 * kk : 2 * kk + 2, N0:CLS],
                        start=(kk == 0),
                        stop=(kk == KT // 2 - 1),
                        perf_mode=dr,
                    )
        else:
            for kt in range(KT):
                for mi in range(gsz):
                    msl = slice(off + mi * P, off + (mi + 1) * P)
                    nc.tensor.matmul(
                        ps_list[mi][:, 0, :],
                        slab[:, kt, msl],
                        anchor_sb[:, kt, 0:N0],
                        start=(kt == 0),
                        stop=(kt == KT - 1),
                    )
                    nc.tensor.matmul(
                        ps_list[mi][:, 1, 0:n1],
                        slab[:, kt, msl],
                        anchor_sb[:, kt, N0:CLS],
                        start=(kt == 0),
                        stop=(kt == KT - 1),
                    )

        for mi in range(gsz):
            m = m0 + mi
            ps = ps_list[mi]
            # flat[:, 0:CLS] covers bank0[0:512] + bank1[0:488]: exactly the
            # computed score columns, so no tail padding is needed
            flat = ps.rearrange("p a b -> p (a b)")
            valid = flat[:, 0:CLS]

            nc.vector.tensor_reduce(
                out=nmx_all[:, m : m + 1],
                in_=valid,
                axis=mybir.AxisListType.X,
                op=mybir.AluOpType.max,
                negate=True,
            )

            # exp(scores - max) with fused per-row sum on the ACT engine;
            # Exp is the only ACT function used -> its table loads once
            expt = scratch.tile([P, CLS], f32, name="expt")
            nc.scalar.activation(
                out=expt,
                in_=valid,
                func=mybir.ActivationFunctionType.Exp,
                bias=nmx_all[:, m : m + 1],
                scale=1.0,
                accum_out=sume_all[:, m : m + 1],
            )

            # s_target = sum_c scores[c] * (iota[c] == target), one DVE pass
            junk = scratch.tile([P, CLS], f32, name="junk")
            nc.vector.scalar_tensor_tensor(
                out=junk,
                in0=iota_f,
                scalar=tgt_sb[:, m : m + 1],
                in1=valid,
                op0=mybir.AluOpType.is_equal,
                op1=mybir.AluOpType.mult,
                accum_out=st_all[:, m : m + 1],
            )

    out_ap = nll.ap()
    nc.sync.dma_start(out=out_ap[:, 0 * MT : 1 * MT], in_=nmx_all)
    nc.sync.dma_start(out=out_ap[:, 1 * MT : 2 * MT], in_=sume_all)
    nc.sync.dma_start(out=out_ap[:, 2 * MT : 3 * MT], in_=st_all)


def _np_mm(mm_dtype: str):
    if mm_dtype in ("bf16", "fp8"):
        import ml_dtypes

        return np.dtype(
            ml_dtypes.bfloat16 if mm_dtype == "bf16" else ml_dtypes.float8_e4m3
        )
    return np.dtype(np.float32)


def prepare_inputs(feature, anchor, _target, mm_dtype: str = MM_DTYPE):
    """Host-side sharding + layout prep. Returns per-core input maps."""
    npdt = _np_mm(mm_dtype)
    clsp = _clsp(mm_dtype)
    feature = np.asarray(feature, dtype=np.float32)
    anchor = np.asarray(anchor, dtype=np.float32)
    tgt_f = np.asarray(_target).astype(np.float32)

    anchorT = np.zeros((FEAT, clsp), dtype=npdt)
    anchorT[:, :CLS] = anchor.T.astype(npdt)  # pad classes stay all-zero
    in_maps = []
    for c in range(NCORES):
        sl = slice(c * BPC, (c + 1) * BPC)
        featT_c = np.ascontiguousarray(feature[sl].T.astype(npdt))  # [FEAT, BPC]
        in_maps.append(
            {
                "featT": featT_c,
                "anchorT": anchorT,
                "tgt": np.ascontiguousarray(tgt_f[sl]),
            }
        )
    return in_maps


_PROGRAM_CACHE: dict = {}


def _get_program(mm_dtype: str, reps: int = 1, variant: str = "hostT") -> bass.Bass:
    key = (mm_dtype, reps, variant)
    nc = _PROGRAM_CACHE.get(key)
    if nc is None:
        if variant == "hostT":
            nc = build_program(mm_dtype, reps=reps)
        elif variant.startswith("loop"):
            nc = build_program(mm_dtype, loop_iters=int(variant[4:]))
        else:
            raise ValueError(variant)
        nc.compile()  # bacc pass pipeline (reg alloc, wait splitting, ...)
        _PROGRAM_CACHE[key] = nc
    return nc


_RUNNER_CACHE: dict = {}


def make_runner(nc: bass.Bass, in_maps):
    """Compile once; return callable that re-executes with device-resident
    inputs (only the tiny donated output zeros are re-created per call)."""
    import jax
    import jax.core
    from jax.experimental.shard_map import shard_map
    from jax.sharding import Mesh, NamedSharding, PartitionSpec

    from concourse import bass2jax, mybir as mb

    bass2jax.install_neuronx_cc_hook()

    partition_name = (
        nc.partition_id_tensor.name if nc.partition_id_tensor else None
    )
    in_names, out_names, out_avals, zero_shapes = [], [], [], []
    for alloc in nc.m.functions[0].allocations:
        if not isinstance(alloc, mb.MemoryLocationSet):
            continue
        name = alloc.memorylocations[0].name
        if alloc.kind == "ExternalInput":
            if name != partition_name:
                in_names.append(name)
        elif alloc.kind == "ExternalOutput":
            shape = tuple(alloc.tensor_shape)
            dtype = mb.dt.np(alloc.dtype)
            out_names.append(name)
            out_avals.append(jax.core.ShapedArray(shape, dtype))
            zero_shapes.append((shape, dtype))
    n_params = len(in_names)
    n_outs = len(out_names)
    all_in_names = list(in_names) + list(out_names)
    if partition_name is not None:
        all_in_names.append(partition_name)

    donate = tuple(range(n_params, n_params + n_outs))

    def _body(*args):
        operands = list(args)
        if partition_name is not None:
            operands.append(bass2jax.partition_id_tensor())
        outs = bass2jax._bass_exec_p.bind(
            *operands,
            out_avals=tuple(out_avals),
            in_names=tuple(all_in_names),
            out_names=tuple(out_names),
            lowering_input_output_aliases=(),
            sim_require_finite=True,
            sim_require_nnan=True,
            nc=nc,
        )
        return tuple(outs)

    devices = jax.devices()[:NCORES]
    mesh = Mesh(np.asarray(devices), ("core",))
    in_specs = (PartitionSpec("core"),) * (n_params + n_outs)
    out_specs = (PartitionSpec("core"),) * n_outs
    sharded = jax.jit(
        shard_map(
            _body, mesh=mesh, in_specs=in_specs, out_specs=out_specs,
            check_rep=False,
        ),
        donate_argnums=donate,
        keep_unused=True,
    )
    sharding = NamedSharding(mesh, PartitionSpec("core"))
    dev_in = [
        jax.device_put(
            np.concatenate([np.asarray(in_maps[c][nm]) for c in range(NCORES)], axis=0),
            sharding,
        )
        for nm in in_names
    ]
    jax.block_until_ready(dev_in)

    def run():
        zeros = [
            np.zeros((NCORES * s[0], *s[1:]), dt) for (s, dt) in zero_shapes
        ]
        outs = sharded(*dev_in, *zeros)
        jax.block_until_ready(outs)
        return {
            nm: np.asarray(outs[i]).reshape(NCORES, *out_avals[i].shape)
            for i, nm in enumerate(out_names)
        }

    return run


def timed_run(in_maps, mm_dtype: str = MM_DTYPE, reps: int = 1, iters: int = 3,
              variant: str = "hostT"):
    """Compile the reps-times-repeated program, return best wall seconds/call."""
    import time

    key = (mm_dtype, reps, variant, id(in_maps))
    runner = _RUNNER_CACHE.get(key)
    if runner is None:
        nc = _get_program(mm_dtype, reps=reps, variant=variant)
        runner = make_runner(nc, in_maps)
        _RUNNER_CACHE[key] = runner
    runner()  # warmup (compile + first exec)
    best = float("inf")
    for _ in range(iters):
        t0 = time.perf_counter()
        runner()
        best = min(best, time.perf_counter() - t0)
    return best


def run_on_cores(in_maps, mm_dtype: str = MM_DTYPE, trace: bool = False):
    from concourse.bass_utils import run_bass_kernel_spmd

    nc = _get_program(mm_dtype)
    res = run_bass_kernel_spmd(nc, in_maps, list(range(NCORES)), trace=trace)
    return res


def finish_nll(stats: np.ndarray) -> np.ndarray:
    """Per-core [P, 3*MT] device stats -> per-row nll [BPC] (float64)."""
    s = stats.astype(np.float64).reshape(P, 3, MT)
    nmx = s[:, 0, :].T.ravel()   # batch row m*128+p <- [p, m]
    sume = s[:, 1, :].T.ravel()
    st = s[:, 2, :].T.ravel()
    return np.log(sume) - nmx - st


def kernel(feature, anchor, _target) -> np.ndarray:
    mm_dtype = MM_DTYPE
    in_maps = prepare_inputs(feature, anchor, _target, mm_dtype)
    res = run_on_cores(in_maps, mm_dtype, trace=os.environ.get("CL_TRACE", "") == "1")
    nll_all = np.concatenate(
        [finish_nll(res.results[c]["nll"]) for c in range(NCORES)]
    )
    if os.environ.get("CL_TRACE", "") == "1" and res.exec_time_ns is not None:
        print(f"HW exec time: {res.exec_time_ns} ns")
    return np.asarray(np.mean(nll_all, dtype=np.float64), dtype=np.float32)
